# revision 25
# baseline (speedup 1.0000x reference)
"""Trainium2 Bass kernel for nn_Attention (cosine-sim attention with null-kv).

Computes, for x [B=4, N=2048, D=1024]:
  xn = LayerNorm(x) * gamma
  q = xn @ Wq; k,v = split(xn @ Wkv); prepend null k/v token
  q = l2norm(q) * q_scale; k = l2norm(k) * k_scale
  scores = (q.k) / sqrt(dh) + mask_bias; attn = softmax(scores)
  out = (attn @ v) @ Wout

Sharding: 8 cores = batch(4) x query-half(2).  Each core gets its batch's
full token set (query half permuted first), computes K/V for all 2048
tokens + null, and attention outputs for its 1024 queries.  No
collectives are needed; the host concatenates the 8 output shards.

Device-side layout notes:
  - xnT (transposed LN output, [dim, tok], bf16) is the hub: Q/K
    projections use weight-stationary matmuls producing qT/kT
    [inner, tok]; V uses xnT-stationary producing natural v [tok, inner].
  - all big matmul operand pairs are bf16 (full PE rate, half the SBUF
    footprint and DMA bytes); PSUM accumulation stays fp32.  Norm/score
    bookkeeping stays f32/f32r.
  - scores are computed transposed, sT[k, q]; softmax needs no max
    subtraction (cosine-sim bounds |logit| <= 8); masking, 1/sqrt(dh) and
    the per-key 1/|k| l2norm factor are folded into the Exp activation's
    scale/bias operands.
  - a ones-column appended to V makes the attn@v matmul also emit the
    softmax denominator (row 64 of each head's output).
  - keys are padded 2049 -> 17*128 with bias -1e4 so all loops are uniform.
  - attention outputs stay SBUF-resident (bf16) for the output projection;
    no DRAM staging round trip.
  - activation-table steering: Ln/Exp both live in the
    natural_log_exp_and_others table set; we shrink the advertised
    contents of the other sets during compile so the table-load insertion
    pass picks that set for both (79 table loads -> ~1).
  - the per-head softmax-division tail (copy/reciprocal/replicate/mul)
    runs on buffers independent of the next head's PSUM accumulator, so
    the PE stays busy across heads (keeps the HAM clock at 8/8).
"""

import os
import sys

sys.path.insert(0, "/opt/trn_rl_repo")

from contextlib import ExitStack

import numpy as np

import concourse.bass as bass
import concourse.mybir as mybir
import concourse.tile as tile
from concourse import bacc
from concourse.bass_utils import run_bass_kernel_spmd

F32 = mybir.dt.float32
F32R = mybir.dt.float32r
BF16 = mybir.dt.bfloat16
AF = mybir.ActivationFunctionType
ALU = mybir.AluOpType

B, N, DIM = 4, 2048, 1024
HEADS, DH = 16, 64
INNER = HEADS * DH
T = 2048          # tokens per core (full batch, query half first)
Q = 1024          # queries per core
KT = 17           # key tiles of 128 (2048 tokens + null + 127 pad)
KPAD = KT * 128   # 2176
DT = DIM // 128   # 8 dim chunks
NPAIR = HEADS // 2
EPS_LN = 1e-5
EPS_L2 = 1e-12
NEG = -10000.0

_CACHE = {}


def _patched_act_tables(orig_fn):
    """Return a get_activation_tables wrapper that hides ln/exp from every
    set except natural_log_exp_and_others, steering the table-load pass to
    the one set that holds both (the emitted set id stays truthful)."""
    def fn(arch):
        t = orig_fn(arch)
        keep = "natural_log_exp_and_others"
        drop = {AF.Ln, AF.Exp}
        return {
            name: (funcs if name == keep else funcs - drop)
            for name, funcs in t.items()
        }
    return fn


def _build_nc():
    nc = bacc.Bacc()

    x_d = nc.declare_dram_parameter("x", [T, DIM], F32, isOutput=False)
    wq_d = nc.declare_dram_parameter("wq", [DIM, INNER], BF16, isOutput=False)
    wkv_d = nc.declare_dram_parameter("wkv", [DIM, 2 * INNER], BF16,
                                      isOutput=False)
    wout_d = nc.declare_dram_parameter("wout", [INNER, DIM], BF16,
                                       isOutput=False)
    gcol_d = nc.declare_dram_parameter("gamma_cols", [128, DT], F32,
                                       isOutput=False)
    bias_d = nc.declare_dram_parameter("bias_cols", [128, KT], F32,
                                       isOutput=False)
    nullk_d = nc.declare_dram_parameter("null_k_cols", [128, NPAIR], BF16,
                                        isOutput=False)
    nullv_d = nc.declare_dram_parameter("null_v_tile", [128, HEADS * 128],
                                        BF16, isOutput=False)
    ones_d = nc.declare_dram_parameter("ones_col", [128, HEADS], BF16,
                                       isOutput=False)
    ident_d = nc.declare_dram_parameter("ident", [128, 128], F32R,
                                        isOutput=False)
    zeros_d = nc.declare_dram_parameter("zeros128", [128, 128], BF16,
                                        isOutput=False)
    onesr_d = nc.declare_dram_parameter("ones_r", [1, 64], BF16, isOutput=False)
    e2_d = nc.declare_dram_parameter("e2", [2, 128], F32R, isOutput=False)
    ks2_d = nc.declare_dram_parameter("k_scale2", [128, 1], F32, isOutput=False)
    esum_d = nc.declare_dram_parameter("esum", [128, 2], F32R, isOutput=False)
    out_d = nc.declare_dram_parameter("out", [Q, DIM], F32, isOutput=True)

    trace_sim = bool(int(os.environ.get("KERNEL_TRACE_SIM", "0")))
    with tile.TileContext(nc, pool_alloc_mode="queue",
                          trace_sim=trace_sim) as tc, ExitStack() as ctx:
        singles = ctx.enter_context(tc.tile_pool(name="singles", bufs=1))
        ident = singles.tile([128, 128], F32R)
        nc.gpsimd.dma_start(out=ident, in_=ident_d[:, :])
        gcols = singles.tile([128, DT], F32)
        nc.gpsimd.dma_start(out=gcols, in_=gcol_d[:, :])
        biasc = singles.tile([128, KT], F32)
        nc.gpsimd.dma_start(out=biasc, in_=bias_d[:, :])
        ks2 = singles.tile([128, 1], F32)
        nc.gpsimd.dma_start(out=ks2, in_=ks2_d[:, :])
        esum = singles.tile([128, 2], F32R)
        nc.gpsimd.dma_start(out=esum, in_=esum_d[:, :])
        e2 = singles.tile([2, 128], F32R)
        nc.gpsimd.dma_start(out=e2, in_=e2_d[:, :])
        ones1 = singles.tile([1, 64], BF16)
        nc.gpsimd.dma_start(out=ones1, in_=onesr_d[:, :])
        eps_ln = singles.tile([128, 1], F32)
        nc.vector.memset(eps_ln, EPS_LN)
        eps_k = singles.tile([128, 1], F32)
        nc.vector.memset(eps_k, 64.0 * EPS_L2)
        eps_q = singles.tile([128, 1], F32)
        nc.vector.memset(eps_q, EPS_L2)

        sc_pool = ctx.enter_context(tc.tile_pool(name="sc", bufs=1))
        inv_kn = [
            sc_pool.tile([128, KT, 2], F32, tag=f"ikn{p}", name=f"ikn{p}")
            for p in range(NPAIR)
        ]
        # attention outputs (normalized, transposed) stay SBUF-resident
        oT_sb = [
            sc_pool.tile([128, Q], BF16, tag=f"oTs{p}", name=f"oTs{p}")
            for p in range(NPAIR)
        ]
        # output-projection + V weights, prefetched early on the idle gpsimd
        # queue
        wo = [
            sc_pool.tile([128, DIM], BF16, tag=f"wo{p}", name=f"wo{p}")
            for p in range(NPAIR)
        ]
        wv = [
            sc_pool.tile([128, 2, 512], BF16, tag=f"wv{dc}", name=f"wv{dc}")
            for dc in range(DT)
        ]

        # xn (LayerNorm output) and v share slots: xn dies at the transpose,
        # v is written by the V projection afterwards.
        with tc.tile_pool(name="vx", bufs=1) as vx_pool, \
             tc.tile_pool(name="xnT", bufs=1) as xnT_pool:
            xnT = [
                xnT_pool.tile([128, T], BF16, tag=f"xnT{d}", name=f"xnT{d}")
                for d in range(DT)
            ]

            # ---------------- P1: LayerNorm (natural layout, in place) ------
            # All 16 x DMAs go out first on the two HWDGE queues so tiles
            # stream in at wire rate; LN is computed in place in the vx slots.
            xn = []
            for t in range(T // 128):
                xnt = vx_pool.tile([128, DIM], F32R, tag=f"vx{t}",
                                   name=f"xn{t}")
                eng = nc.sync if t % 2 == 0 else nc.scalar
                eng.dma_start(out=xnt,
                              in_=x_d[t * 128:(t + 1) * 128, :].bitcast(F32R))
                xn.append(xnt)
            # prefetch V/output-projection weights AFTER the x tiles so the
            # LN-gating x loads get the HBM wire first
            for dc in range(DT):
                nc.gpsimd.dma_start(
                    out=wv[dc],
                    in_=wkv_d[dc * 128:(dc + 1) * 128,
                              INNER:2 * INNER].rearrange(
                        "p (n c) -> p n c", n=2))
            for p in range(NPAIR):
                nc.gpsimd.dma_start(out=wo[p],
                                    in_=wout_d[p * 128:(p + 1) * 128, :])
            with tc.tile_pool(name="ln_tmp", bufs=3) as lnp, \
                 tc.tile_pool(name="tp_ps", bufs=4, space="PSUM") as tpp:
                for t in range(T // 128):
                    xnt = xn[t]
                    xg = xnt.bitcast(F32).rearrange("p (s d) -> p s d", s=2)
                    stats = lnp.tile([128, 2, 6], F32, tag="stats", name="stats")
                    nc.vector.bn_stats(out=stats[:, 0, :], in_=xg[:, 0, :])
                    nc.vector.bn_stats(out=stats[:, 1, :], in_=xg[:, 1, :])
                    mv = lnp.tile([128, 2], F32, tag="mv", name="mv")
                    nc.vector.bn_aggr(out=mv, in_=stats)
                    rstd = lnp.tile([128, 1], F32, tag="rstd", name="rstd")
                    nc.scalar.activation(out=rstd, in_=mv[:, 1:2], func=AF.Ln,
                                         bias=eps_ln, scale=1.0)
                    nc.scalar.activation(out=rstd, in_=rstd, func=AF.Exp,
                                         bias=0.0, scale=-0.5)
                    nmb = lnp.tile([128, 1], F32, tag="nmb", name="nmb")
                    nc.vector.tensor_scalar(out=nmb, in0=mv[:, 0:1],
                                            scalar1=rstd, scalar2=-1.0,
                                            op0=ALU.mult, op1=ALU.mult)
                    nc.scalar.activation(out=xnt, in_=xnt.bitcast(F32),
                                         func=AF.Identity,
                                         bias=nmb, scale=rstd)

                # -------- P2: transpose xn -> xnT (gamma fused), g-outer ------
                for g in range(4):  # groups of 512 tokens
                    for d in range(DT):
                        ps = tpp.tile([128, 512], F32, tag="tps", name="tps")
                        for j in range(4):
                            nc.tensor.transpose(
                                out=ps[:, j * 128:(j + 1) * 128].bitcast(F32R),
                                in_=xn[g * 4 + j][:, d * 128:(d + 1) * 128],
                                identity=ident,
                            )
                        nc.vector.tensor_scalar_mul(
                            out=xnT[d][:, g * 512:(g + 1) * 512], in0=ps,
                            scalar1=gcols[:, d:d + 1])

            # -------- P3: V projection (SBUF-resident, reuses xn slots) ------
            # v is padded to 128 columns per head (cols DH+1..127 zero) so
            # the attnv weight loads hit the fast 128-column path
            v = [
                vx_pool.tile([128, HEADS, 128], BF16, tag=f"vx{i}",
                             name=f"v{i}")
                for i in range(KT)
            ]
            nc.sync.dma_start(out=v[16].rearrange("p h d -> p (h d)"),
                              in_=nullv_d[:, :])
            with tc.tile_pool(name="v_ps", bufs=4, space="PSUM") as vpp:
                for t in range(T // 128):
                    vt = v[t]
                    nc.vector.memset(vt[:, :, DH + 1:128], 0.0)
                    nc.sync.dma_start(
                        out=vt[:, :, DH:DH + 1],
                        in_=ones_d[:, :].rearrange("p (h o) -> p h o", o=1))
                    for nn in range(2):
                        ps = vpp.tile([128, 512], F32, tag="vps", name="vps")
                        for dc in range(DT):
                            nc.tensor.matmul(
                                out=ps,
                                lhsT=xnT[dc][:, t * 128:(t + 1) * 128],
                                rhs=wv[dc][:, nn, :],
                                start=(dc == 0), stop=(dc == DT - 1))
                        nc.vector.tensor_copy(
                            out=vt[:, nn * 8:(nn + 1) * 8, 0:DH],
                            in_=ps.rearrange("p (h d) -> p h d", d=DH))

            # ------ P4+P5: per-pair K/Q projection interleaved with attention
            with tc.tile_pool(name="kq_sb", bufs=2) as kqsb, \
                 tc.tile_pool(name="wkq", bufs=2) as wkp, \
                 tc.tile_pool(name="sqp", bufs=1) as sqp, \
                 tc.tile_pool(name="smp", bufs=1) as smp, \
                 tc.tile_pool(name="expp", bufs=4) as expp, \
                 tc.tile_pool(name="stage", bufs=2) as stp, \
                 tc.tile_pool(name="kq_ps", bufs=2, space="PSUM") as kqps, \
                 tc.tile_pool(name="s_ps", bufs=2, space="PSUM") as sps, \
                 tc.tile_pool(name="o_ps", bufs=1, space="PSUM") as ops:

                def load_kT(p):
                    kT = kqsb.tile([128, KPAD], BF16, tag="kT", name=f"kT{p}")
                    nc.sync.dma_start(out=kT[:, 2048:KPAD], in_=zeros_d[:, :])
                    nc.sync.dma_start(out=kT[:, 2048:2049],
                                      in_=nullk_d[:, p:p + 1])
                    return kT

                def load_w(p):
                    wk = wkp.tile([128, DT, 128], BF16, tag="w", name=f"wk{p}")
                    nc.sync.dma_start(
                        out=wk,
                        in_=wkv_d[:, p * 128:(p + 1) * 128].rearrange(
                            "(dc p2) m -> p2 dc m", p2=128))
                    wq = wkp.tile([128, DT, 128], BF16, tag="w", name=f"wq{p}")
                    nc.sync.dma_start(
                        out=wq,
                        in_=wq_d[:, p * 128:(p + 1) * 128].rearrange(
                            "(dc p2) m -> p2 dc m", p2=128))
                    return wk, wq

                kT_t = [None] * NPAIR
                qTr_t = [None] * NPAIR

                def proj_units(p):
                    """Projection of pair p as a list of closures.  These are
                    dripped between attention iterations of pair p-1 so the
                    PE FIFO interleaves projection matmuls into the
                    exp-gated slack of the attention stream."""
                    state = {}

                    def u_load():
                        kT_t[p] = load_kT(p)
                        state["w"] = load_w(p)
                        state["sq"] = sqp.tile([128, KPAD], F32R, tag="sq",
                                               name=f"sqk{p}")

                    def u_kchunk(c):
                        kT = kT_t[p]
                        wk = state["w"][0]
                        sq = state["sq"]
                        ps = kqps.tile([128, 512], F32, tag="kqps", name="kqps")
                        for dc in range(DT):
                            nc.tensor.matmul(
                                out=ps, lhsT=wk[:, dc, :],
                                rhs=xnT[dc][:, c * 512:(c + 1) * 512],
                                start=(dc == 0), stop=(dc == DT - 1))
                        sl = slice(c * 512, (c + 1) * 512)
                        nc.vector.tensor_copy(out=kT[:, sl], in_=ps)
                        nc.vector.tensor_mul(out=sq[:, sl], in0=kT[:, sl],
                                             in1=kT[:, sl])

                    def u_knorm():
                        kT = kT_t[p]
                        sq = state["sq"]
                        nc.vector.tensor_mul(out=sq[:, 2048:KPAD],
                                             in0=kT[:, 2048:KPAD],
                                             in1=kT[:, 2048:KPAD])
                        n2t = kqps.tile([128, 512], F32, tag="kqps", name="n2k")
                        n2 = n2t[:, 0:2 * KT].rearrange("p (k h) -> p k h", h=2)
                        for i in range(KT):
                            nc.tensor.matmul(out=n2[:, i, :],
                                             lhsT=sq[:, i * 128:(i + 1) * 128],
                                             rhs=esum, start=True, stop=True)
                        # 1/(8|k|) = exp(-0.5 ln(64 n2 + eps)); Ln/Exp share
                        # one ACT table set (steered) so no table thrash
                        kn = smp.tile([128, KT, 2], F32, tag="kn", name="kn")
                        nc.scalar.activation(out=kn, in_=n2, func=AF.Ln,
                                             bias=eps_k, scale=64.0)
                        nc.scalar.activation(out=inv_kn[p], in_=kn,
                                             func=AF.Exp, bias=0.0, scale=-0.5)
                        nc.vector.tensor_scalar_mul(out=kT, in0=kT,
                                                    scalar1=ks2)

                    def u_qchunk(c):
                        if c == 0:
                            qTr_t[p] = kqsb.tile([128, Q], BF16, tag="qTr",
                                                 name=f"qTr{p}")
                            state["sqq"] = sqp.tile([128, Q], F32R, tag="sq",
                                                    name=f"sqq{p}")
                            state["qn01"] = smp.tile([2, Q], F32R, tag="qn01",
                                                     name="qn01")
                        qTr = qTr_t[p]
                        wq = state["w"][1]
                        sqq = state["sqq"]
                        ps = kqps.tile([128, 512], F32, tag="kqps", name="kqps")
                        for dc in range(DT):
                            nc.tensor.matmul(
                                out=ps, lhsT=wq[:, dc, :],
                                rhs=xnT[dc][:, c * 512:(c + 1) * 512],
                                start=(dc == 0), stop=(dc == DT - 1))
                        sl = slice(c * 512, (c + 1) * 512)
                        nc.vector.tensor_copy(out=qTr[:, sl], in_=ps)
                        nc.vector.tensor_mul(out=sqq[:, sl], in0=qTr[:, sl],
                                             in1=qTr[:, sl])

                    def u_qnorm(c):
                        qTr = qTr_t[p]
                        sqq = state["sqq"]
                        qn01 = state["qn01"]
                        sl = slice(c * 512, (c + 1) * 512)
                        n2qt = kqps.tile([128, 512], F32, tag="kqps",
                                         name="n2q")
                        n2q = n2qt[0:2, :]
                        nc.tensor.matmul(out=n2q, lhsT=esum,
                                         rhs=sqq[:, sl], start=True, stop=True)
                        nc.scalar.activation(out=n2q, in_=n2q, func=AF.Ln,
                                             bias=eps_q[0:2, :], scale=1.0)
                        nc.scalar.activation(out=qn01[:, sl], in_=n2q,
                                             func=AF.Exp, bias=0.0, scale=-0.5)
                        # replicate 1/|q| across partitions: e2.T @ qn01
                        qrep = kqps.tile([128, 512], F32, tag="kqps",
                                         name="qrep")
                        nc.tensor.matmul(out=qrep, lhsT=e2, rhs=qn01[:, sl],
                                         start=True, stop=True)
                        nc.vector.tensor_mul(out=qTr[:, sl], in0=qTr[:, sl],
                                             in1=qrep)

                    return ([u_load] +
                            [lambda c=c: u_kchunk(c) for c in range(4)] +
                            [u_knorm] +
                            [lambda c=c: u_qchunk(c) for c in range(2)] +
                            [lambda c=c: u_qnorm(c) for c in range(2)])

                def make_tail(p, h, num, den):
                    """Deferred tail: replicate 1/den across partitions and
                    scale.  Emitted a few iterations into the NEXT head so the
                    rep matmuls never wait on the reciprocal in the PE FIFO."""
                    def run():
                        for c in range(2):
                            sl = slice(c * 512, (c + 1) * 512)
                            rep = kqps.tile([128, 512], F32, tag="kqps",
                                            name="rep")
                            nc.tensor.matmul(out=rep[0:64, :],
                                             lhsT=ones1,
                                             rhs=den[:, sl], start=True,
                                             stop=True)
                            nc.vector.tensor_mul(
                                out=oT_sb[p][h * 64:(h + 1) * 64, sl],
                                in0=num[:, sl], in1=rep[0:64, :])
                    return run

                deferred = []
                for u in proj_units(0):
                    u()
                for p in range(NPAIR):
                    kT = kT_t[p]
                    qTr = qTr_t[p]
                    pending = proj_units(p + 1) if p + 1 < NPAIR else []
                    # drip schedule: one projection unit every few attention
                    # iterations (34 iterations, ~10 units); deferred
                    # normalization tails interleave on a different phase
                    drip_every = 3
                    it = 0
                    def emit_scores(h, i):
                        sT = sps.tile([128, Q], F32, tag="sT", name="sT")
                        for c in range(2):
                            nc.tensor.matmul(
                                out=sT[:, c * 512:(c + 1) * 512],
                                lhsT=kT[h * 64:(h + 1) * 64,
                                        i * 128:(i + 1) * 128],
                                rhs=qTr[h * 64:(h + 1) * 64,
                                        c * 512:(c + 1) * 512],
                                start=True, stop=True)
                        return sT

                    for h in range(2):
                        hg = 2 * p + h
                        oT = ops.tile([128, Q], F32, tag="oT", name="oT")
                        for i in range(KT):
                            sT = emit_scores(h, i)
                            ex = expp.tile([128, Q], BF16, tag="ex", name="ex")
                            nc.scalar.activation(out=ex, in_=sT, func=AF.Exp,
                                                 bias=biasc[:, i:i + 1],
                                                 scale=inv_kn[p][:, i, h:h + 1])
                            for c in range(2):
                                nc.tensor.matmul(
                                    out=oT[:, c * 512:(c + 1) * 512],
                                    lhsT=v[i][:, hg, :],
                                    rhs=ex[:, c * 512:(c + 1) * 512],
                                    start=(i == 0), stop=(i == KT - 1))
                            it += 1
                            if deferred and it % drip_every == 1:
                                deferred.pop(0)()
                            if pending and it % drip_every == 0:
                                pending.pop(0)()
                        # inline tail: copy num+den out of PSUM immediately
                        # (releases oT for the next head after ~1us) and start
                        # the fast reciprocal on the SBUF copy (DVE-only, does
                        # not block the PE FIFO).
                        num = stp.tile([64, Q], BF16, tag="num", name="num")
                        nc.vector.tensor_copy(out=num, in_=oT[0:64, :])
                        draw = stp.tile([1, Q], F32, tag="draw", name="draw",
                                        bufs=1)
                        nc.vector.tensor_copy(out=draw, in_=oT[64:65, :])
                        dscr = stp.tile([1, Q], F32, tag="dscr", name="dscr",
                                        bufs=1)
                        nc.vector.reciprocal_approx_fast(out=dscr, in_=draw)
                        den = stp.tile([1, Q], BF16, tag="den", name="den")
                        nc.vector.tensor_copy(out=den, in_=dscr)
                        deferred.append(make_tail(p, h, num, den))
                    for u in pending:
                        u()
                for u in deferred:
                    u()

        # ---------------- P6: output projection ----------------
        with tc.tile_pool(name="ob", bufs=4) as obp, \
             tc.tile_pool(name="oc_ps", bufs=6, space="PSUM") as ocp:
            for t in range(Q // 128):
                ob = obp.tile([128, DIM], F32, tag="ob", name="ob")
                for nn in range(2):
                    ps = ocp.tile([128, 512], F32, tag="ocps", name="ocps")
                    for p in range(NPAIR):
                        nc.tensor.matmul(
                            out=ps,
                            lhsT=oT_sb[p][:, t * 128:(t + 1) * 128],
                            rhs=wo[p][:, nn * 512:(nn + 1) * 512],
                            start=(p == 0), stop=(p == NPAIR - 1))
                    nc.vector.tensor_copy(out=ob[:, nn * 512:(nn + 1) * 512],
                                          in_=ps)
                nc.sync.dma_start(out=out_d[t * 128:(t + 1) * 128, :], in_=ob)

    # Steer the ACT-table insertion pass toward the ln+exp set, then
    # restore the original lookup.
    orig = bacc.get_activation_tables
    bacc.get_activation_tables = _patched_act_tables(orig)
    try:
        nc.compile()
    finally:
        bacc.get_activation_tables = orig
    return nc


def _host_prep(x, context_mask, gamma, null_kv, Wq, Wkv, q_scale, k_scale, Wout):
    """Build per-core input maps (host-side marshalling only)."""
    try:
        import ml_dtypes
        bf16 = ml_dtypes.bfloat16
    except ImportError:  # jax ships ml_dtypes; fall back via numpy name
        bf16 = np.dtype("bfloat16")
    x = np.ascontiguousarray(np.asarray(x, dtype=np.float32))
    mask = np.asarray(context_mask).astype(bool)
    gamma = np.asarray(gamma, dtype=np.float32)
    null_kv = np.asarray(null_kv, dtype=np.float32)
    Wq = np.ascontiguousarray(np.asarray(Wq, dtype=np.float32).astype(bf16))
    Wkv = np.ascontiguousarray(np.asarray(Wkv, dtype=np.float32).astype(bf16))
    q_scale = np.asarray(q_scale, dtype=np.float32)
    k_scale = np.asarray(k_scale, dtype=np.float32)
    Wout = np.ascontiguousarray(np.asarray(Wout, dtype=np.float32).astype(bf16))

    gcols = np.ascontiguousarray(gamma.reshape(DT, 128).T)
    nullk = np.ascontiguousarray(
        null_kv[0].reshape(HEADS * DH).reshape(NPAIR, 128).T.astype(bf16))
    nullv_tile = np.zeros((128, HEADS, 128), dtype=np.float32)
    nullv_tile[0, :, 0:DH] = null_kv[1].reshape(HEADS, DH)
    nullv_tile[:, :, DH] = 1.0
    nullv_tile = np.ascontiguousarray(
        nullv_tile.reshape(128, HEADS * 128).astype(bf16))
    ones_col = np.ones((128, HEADS), dtype=bf16)
    ident = np.eye(128, dtype=np.float32)
    ones_r = np.ones((1, 64), dtype=bf16)
    e2 = np.zeros((2, 128), dtype=np.float32)
    e2[0, 0:64] = q_scale
    e2[1, 64:128] = q_scale
    zeros128 = np.zeros((128, 128), dtype=bf16)
    ks2 = np.ascontiguousarray(np.tile(k_scale, 2).reshape(128, 1))
    esum = np.zeros((128, 2), dtype=np.float32)
    esum[0:64, 0] = 1.0
    esum[64:128, 1] = 1.0

    in_maps = []
    for c in range(8):
        b, qh = c // 2, c % 2
        if qh == 0:
            xp = x[b]
            mb = mask[b]
        else:
            perm = np.concatenate([np.arange(Q, T), np.arange(0, Q)])
            xp = np.ascontiguousarray(x[b][perm])
            mb = mask[b][perm]
        bias_vec = np.full(KPAD, NEG, dtype=np.float32)
        bias_vec[0:T] = np.where(mb, 0.0, NEG)
        bias_vec[T] = 0.0  # null token always attendable
        bias_cols = np.ascontiguousarray(bias_vec.reshape(KT, 128).T)
        in_maps.append({
            "x": xp,
            "wq": Wq,
            "wkv": Wkv,
            "wout": Wout,
            "gamma_cols": gcols,
            "bias_cols": bias_cols,
            "null_k_cols": nullk,
            "null_v_tile": nullv_tile,
            "ones_col": ones_col,
            "ident": ident,
            "ones_r": ones_r,
            "e2": e2,
            "zeros128": zeros128,
            "k_scale2": ks2,
            "esum": esum,
        })
    return in_maps


def kernel(x, context_mask, gamma, null_kv, Wq, Wkv, q_scale, k_scale, Wout):
    if "nc" not in _CACHE:
        _CACHE["nc"] = _build_nc()
    nc = _CACHE["nc"]
    in_maps = _host_prep(x, context_mask, gamma, null_kv, Wq, Wkv,
                         q_scale, k_scale, Wout)
    res = run_bass_kernel_spmd(nc, in_maps, core_ids=list(range(8)))
    _CACHE["last_result"] = res
    out = np.empty((B, N, DIM), dtype=np.float32)
    for c in range(8):
        b, qh = c // 2, c % 2
        out[b, qh * Q:(qh + 1) * Q, :] = res.results[c]["out"]
    return out


def bench(in_maps, warmup=3, iters=150):
    """Steady-state per-invocation timing of the compiled NEFF on 8 cores.

    Mirrors run_bass_via_pjrt's multi-core path but jits ONCE (no output
    donation; the kernel writes every output element) and places inputs
    pre-sharded across the 8 cores (NamedSharding) so repeated calls
    measure dispatch+execute only — no per-call re-scatter.
    Returns (pipelined_ns, blocking_ns) per invocation.
    """
    import time

    import jax
    from jax.sharding import NamedSharding
    from concourse import bass2jax
    from concourse.bass2jax import (Mesh, PartitionSpec, shard_map,
                                    _bass_exec_p)
    import concourse.mybir as mybir_

    if "nc" not in _CACHE:
        _CACHE["nc"] = _build_nc()
    nc = _CACHE["nc"]
    bass2jax.install_neuronx_cc_hook()

    partition_name = (nc.partition_id_tensor.name
                      if nc.partition_id_tensor else None)
    in_names, out_names, out_avals, zero_outs = [], [], [], []
    for alloc in nc.m.functions[0].allocations:
        if not isinstance(alloc, mybir_.MemoryLocationSet):
            continue
        name = alloc.memorylocations[0].name
        if alloc.kind == "ExternalInput":
            if name != partition_name:
                in_names.append(name)
        elif alloc.kind == "ExternalOutput":
            out_names.append(name)
            shape = tuple(alloc.tensor_shape)
            dtype = mybir_.dt.np(alloc.dtype)
            out_avals.append(jax.core.ShapedArray(shape, dtype))
            zero_outs.append(np.zeros(shape, dtype))

    n_cores = 8
    bind_names = list(in_names) + list(out_names)
    if partition_name is not None:
        bind_names.append(partition_name)

    def _body(*args):
        operands = list(args)
        if partition_name is not None:
            operands.append(bass2jax.partition_id_tensor())
        outs = _bass_exec_p.bind(
            *operands,
            out_avals=tuple(out_avals),
            in_names=tuple(bind_names),
            out_names=tuple(out_names),
            lowering_input_output_aliases=(),
            sim_require_finite=False,
            sim_require_nnan=False,
            nc=nc,
        )
        return tuple(outs)

    devices = jax.devices()[:n_cores]
    mesh = Mesh(np.asarray(devices), ("core",))
    n_ops = len(in_names) + len(out_names)
    fn = jax.jit(shard_map(
        _body, mesh=mesh,
        in_specs=(PartitionSpec("core"),) * n_ops,
        out_specs=(PartitionSpec("core"),) * len(out_names),
        check_rep=False), keep_unused=True)

    concat_in = [
        np.concatenate([np.asarray(in_maps[c][k]) for c in range(n_cores)],
                       axis=0)
        for k in in_names
    ] + [np.concatenate([z] * n_cores, axis=0) for z in zero_outs]
    sharding = NamedSharding(mesh, PartitionSpec("core"))
    dev_in = [jax.device_put(a, sharding) for a in concat_in]
    for _ in range(warmup):
        jax.block_until_ready(fn(*dev_in))

    # The axon tunnel adds noisy per-call client overhead; take the best of
    # several pipelined trials to measure sustainable device throughput.
    trials = []
    for _ in range(3):
        t0 = time.perf_counter()
        outs = [fn(*dev_in) for _ in range(iters)]
        jax.block_until_ready(outs)
        t1 = time.perf_counter()
        trials.append((t1 - t0) / iters * 1e9)
    pipelined_ns = min(trials)

    t0 = time.perf_counter()
    for _ in range(20):
        jax.block_until_ready(fn(*dev_in))
    t1 = time.perf_counter()
    blocking_ns = (t1 - t0) / 20 * 1e9
    return pipelined_ns, blocking_ns
